# revision 3
# baseline (speedup 1.0000x reference)
"""Category-equality Gram matrix kernel for TRN2.

out[i, j] = 1.0 if Z[i] == Z[j] else 0.0, Z: [16384] int32 labels in [0, 64).

Row-parallel across 8 NeuronCores: core i computes rows [i*2048, (i+1)*2048).
Per core: DVE tensor_scalar(is_equal) compares a partition-broadcast copy of Z
(f32) against the per-partition row label, producing [128, CHUNK] f32 tiles
streamed to DRAM with large HWDGE DMAs. The kernel is output-write bound
(~128 MiB/core); DVE compute (~138 us) hides under the HBM writes (~370 us).
"""

import numpy as np

import concourse.tile as tile
from concourse import bacc, mybir
from concourse.bass_utils import run_bass_kernel_spmd

N = 16384          # number of labels / output dim
M = 8              # cores
RPC = N // M       # 2048 rows per core
P = 128            # SBUF partitions
T = RPC // P       # 16 row tiles per core
CHUNK = 8192       # output column chunk per DVE op / DMA store
NCH = N // CHUNK

_NC_CACHE = None


def _build_nc():
    nc = bacc.Bacc("TRN2", target_bir_lowering=False, debug=False, num_devices=M)
    zb = nc.dram_tensor("zb", [P, N], mybir.dt.float32, kind="ExternalInput").ap()
    zr = nc.dram_tensor("zr", [P, T], mybir.dt.float32, kind="ExternalInput").ap()
    out = nc.dram_tensor("out", [RPC, N], mybir.dt.float32, kind="ExternalOutput").ap()

    with tile.TileContext(nc) as tc:
        with tc.tile_pool(name="zp", bufs=NCH) as zp, \
             tc.tile_pool(name="ep", bufs=3) as ep, \
             tc.tile_pool(name="rp", bufs=1) as rp:
            zrt = rp.tile([P, T], mybir.dt.float32)
            nc.sync.dma_start(zrt[:], zr[:, :])
            # all input loads upfront: no mid-kernel load queues behind
            # output stores on the HWDGE ring
            zchunks = []
            for c in range(NCH):
                zchunk = zp.tile([P, CHUNK], mybir.dt.float32)
                nc.sync.dma_start(zchunk[:], zb[:, c * CHUNK:(c + 1) * CHUNK])
                zchunks.append(zchunk)
            for c in range(NCH):
                for t in range(T):
                    eq = ep.tile([P, CHUNK], mybir.dt.float32)
                    nc.vector.tensor_scalar(
                        eq[:],
                        zchunks[c][:],
                        zrt[:, t:t + 1],
                        None,
                        mybir.AluOpType.is_equal,
                    )
                    nc.sync.dma_start(
                        out[t * P:(t + 1) * P, c * CHUNK:(c + 1) * CHUNK], eq[:]
                    )
    nc.compile()
    return nc


def _get_nc():
    global _NC_CACHE
    if _NC_CACHE is None:
        _NC_CACHE = _build_nc()
    return _NC_CACHE


def _in_maps(Z: np.ndarray) -> list[dict[str, np.ndarray]]:
    zf = Z.astype(np.float32).reshape(-1)
    zb = np.ascontiguousarray(np.broadcast_to(zf[None, :], (P, N)))
    maps = []
    for i in range(M):
        # zr[p, t] = label of row (i*RPC + t*P + p)
        zr_i = np.ascontiguousarray(zf[i * RPC:(i + 1) * RPC].reshape(T, P).T)
        maps.append({"zb": zb, "zr": zr_i})
    return maps


def kernel(Z: np.ndarray, **_ignored) -> np.ndarray:
    Z = np.asarray(Z).reshape(-1)
    assert Z.shape == (N,), Z.shape
    nc = _get_nc()
    res = run_bass_kernel_spmd(nc, _in_maps(Z), list(range(M)))
    return np.concatenate([res.results[i]["out"] for i in range(M)], axis=0)


# revision 5
# speedup vs baseline: 1.0030x; 1.0030x over previous
"""Category-equality Gram matrix kernel for TRN2.

out[i, j] = 1.0 if Z[i] == Z[j] else 0.0, Z: [16384] int32 labels in [0, 64).

Row-parallel across 8 NeuronCores: core i computes rows [i*2048, (i+1)*2048).
Per core: DVE tensor_scalar(is_equal) compares a partition-broadcast copy of Z
(f32) against the per-partition row label, producing [128, CHUNK] f32 tiles
streamed to DRAM with large HWDGE DMAs. The kernel is output-write bound
(~128 MiB/core); DVE compute (~138 us) hides under the HBM writes (~370 us).
"""

import numpy as np

import concourse.tile as tile
from concourse import bacc, mybir
from concourse.bass_utils import run_bass_kernel_spmd

N = 16384          # number of labels / output dim
M = 8              # cores
RPC = N // M       # 2048 rows per core
P = 128            # SBUF partitions
T = RPC // P       # 16 row tiles per core
CHUNK = 8192       # output column chunk per DVE op / DMA store
NCH = N // CHUNK

_NC_CACHE = None


def _build_nc():
    nc = bacc.Bacc("TRN2", target_bir_lowering=False, debug=False, num_devices=M)
    # uint8 labels (values 0-63): 2 MiB broadcast read instead of 8 MiB f32
    zb = nc.dram_tensor("zb", [P, N], mybir.dt.uint8, kind="ExternalInput").ap()
    zr = nc.dram_tensor("zr", [P, T], mybir.dt.float32, kind="ExternalInput").ap()
    out = nc.dram_tensor("out", [RPC, N], mybir.dt.float32, kind="ExternalOutput").ap()

    with tile.TileContext(nc) as tc:
        with tc.tile_pool(name="zp", bufs=NCH) as zp, \
             tc.tile_pool(name="ep", bufs=4) as ep, \
             tc.tile_pool(name="rp", bufs=1) as rp:
            zrt = rp.tile([P, T], mybir.dt.float32)
            nc.sync.dma_start(zrt[:], zr[:, :])
            # all input loads upfront: no mid-kernel load queues behind
            # output stores on the HWDGE ring
            zchunks = []
            for c in range(NCH):
                zchunk = zp.tile([P, CHUNK], mybir.dt.uint8)
                nc.sync.dma_start(zchunk[:], zb[:, c * CHUNK:(c + 1) * CHUNK])
                zchunks.append(zchunk)
            for c in range(NCH):
                for t in range(T):
                    eq = ep.tile([P, CHUNK], mybir.dt.float32)
                    nc.vector.tensor_scalar(
                        eq[:],
                        zchunks[c][:],
                        zrt[:, t:t + 1],
                        None,
                        mybir.AluOpType.is_equal,
                    )
                    nc.sync.dma_start(
                        out[t * P:(t + 1) * P, c * CHUNK:(c + 1) * CHUNK], eq[:]
                    )
    nc.compile()
    return nc


def _get_nc():
    global _NC_CACHE
    if _NC_CACHE is None:
        _NC_CACHE = _build_nc()
    return _NC_CACHE


def _in_maps(Z: np.ndarray) -> list[dict[str, np.ndarray]]:
    zflat = Z.reshape(-1)
    zb = np.ascontiguousarray(
        np.broadcast_to(zflat.astype(np.uint8)[None, :], (P, N))
    )
    zf = zflat.astype(np.float32)
    maps = []
    for i in range(M):
        # zr[p, t] = label of row (i*RPC + t*P + p)
        zr_i = np.ascontiguousarray(zf[i * RPC:(i + 1) * RPC].reshape(T, P).T)
        maps.append({"zb": zb, "zr": zr_i})
    return maps


def kernel(Z: np.ndarray, **_ignored) -> np.ndarray:
    Z = np.asarray(Z).reshape(-1)
    assert Z.shape == (N,), Z.shape
    nc = _get_nc()
    res = run_bass_kernel_spmd(nc, _in_maps(Z), list(range(M)))
    return np.concatenate([res.results[i]["out"] for i in range(M)], axis=0)


# revision 6
# speedup vs baseline: 1.0053x; 1.0022x over previous
"""Category-equality Gram matrix kernel for TRN2.

out[i, j] = 1.0 if Z[i] == Z[j] else 0.0, Z: [16384] int32 labels in [0, 64).

Row-parallel across 8 NeuronCores: core i computes rows [i*2048, (i+1)*2048).
Per core: DVE tensor_scalar(is_equal) compares a partition-broadcast copy of Z
(f32) against the per-partition row label, producing [128, CHUNK] f32 tiles
streamed to DRAM with large HWDGE DMAs. The kernel is output-write bound
(~128 MiB/core); DVE compute (~138 us) hides under the HBM writes (~370 us).
"""

import numpy as np

import concourse.tile as tile
from concourse import bacc, mybir
from concourse.bass_utils import run_bass_kernel_spmd

N = 16384          # number of labels / output dim
M = 8              # cores
RPC = N // M       # 2048 rows per core
P = 128            # SBUF partitions
T = RPC // P       # 16 row tiles per core
CHUNK = 8192       # output column chunk per DVE op / DMA store
NCH = N // CHUNK

_NC_CACHE = None


def _build_nc():
    nc = bacc.Bacc("TRN2", target_bir_lowering=False, debug=False, num_devices=M)
    # uint8 labels (values 0-63): 2 MiB broadcast read instead of 8 MiB f32
    zb = nc.dram_tensor("zb", [P, N], mybir.dt.uint8, kind="ExternalInput").ap()
    zr = nc.dram_tensor("zr", [P, T], mybir.dt.float32, kind="ExternalInput").ap()
    out = nc.dram_tensor("out", [RPC, N], mybir.dt.float32, kind="ExternalOutput").ap()

    FIRST = 2048   # small first tile so the store stream starts early
    REST = N - FIRST

    with tile.TileContext(nc) as tc:
        with tc.tile_pool(name="zp", bufs=2) as zp, \
             tc.tile_pool(name="ep", bufs=2) as ep, \
             tc.tile_pool(name="e0p", bufs=1) as e0p, \
             tc.tile_pool(name="rp", bufs=1) as rp:
            zrt = rp.tile([P, T], mybir.dt.float32)
            # zr on the scalar HWDGE ring: overlaps with za/zb loads on sync
            nc.scalar.dma_start(zrt[:], zr[:, :])
            za = zp.tile([P, FIRST], mybir.dt.uint8, tag="za")
            nc.sync.dma_start(za[:], zb[:, 0:FIRST])
            zb2 = zp.tile([P, REST], mybir.dt.uint8, tag="zb2")
            nc.sync.dma_start(zb2[:], zb[:, FIRST:N])

            # t=0: two stores, the first one small to prime the pipeline
            e0 = e0p.tile([P, FIRST], mybir.dt.float32)
            nc.vector.tensor_scalar(
                e0[:], za[:], zrt[:, 0:1], None, mybir.AluOpType.is_equal
            )
            nc.sync.dma_start(out[0:P, 0:FIRST], e0[:])
            e1 = ep.tile([P, N], mybir.dt.float32, tag="eq")
            nc.vector.tensor_scalar(
                e1[:, 0:REST], zb2[:], zrt[:, 0:1], None, mybir.AluOpType.is_equal
            )
            nc.sync.dma_start(out[0:P, FIRST:N], e1[:, 0:REST])

            # t>=1: one full-width 8 MiB store per row tile
            for t in range(1, T):
                eq = ep.tile([P, N], mybir.dt.float32, tag="eq")
                nc.vector.tensor_scalar(
                    eq[:, 0:FIRST], za[:], zrt[:, t:t + 1], None,
                    mybir.AluOpType.is_equal,
                )
                nc.vector.tensor_scalar(
                    eq[:, FIRST:N], zb2[:], zrt[:, t:t + 1], None,
                    mybir.AluOpType.is_equal,
                )
                nc.sync.dma_start(out[t * P:(t + 1) * P, :], eq[:])
    nc.compile()
    return nc


def _get_nc():
    global _NC_CACHE
    if _NC_CACHE is None:
        _NC_CACHE = _build_nc()
    return _NC_CACHE


def _in_maps(Z: np.ndarray) -> list[dict[str, np.ndarray]]:
    zflat = Z.reshape(-1)
    zb = np.ascontiguousarray(
        np.broadcast_to(zflat.astype(np.uint8)[None, :], (P, N))
    )
    zf = zflat.astype(np.float32)
    maps = []
    for i in range(M):
        # zr[p, t] = label of row (i*RPC + t*P + p)
        zr_i = np.ascontiguousarray(zf[i * RPC:(i + 1) * RPC].reshape(T, P).T)
        maps.append({"zb": zb, "zr": zr_i})
    return maps


def kernel(Z: np.ndarray, **_ignored) -> np.ndarray:
    Z = np.asarray(Z).reshape(-1)
    assert Z.shape == (N,), Z.shape
    nc = _get_nc()
    res = run_bass_kernel_spmd(nc, _in_maps(Z), list(range(M)))
    return np.concatenate([res.results[i]["out"] for i in range(M)], axis=0)


# revision 7
# speedup vs baseline: 1.2101x; 1.2038x over previous
"""Category-equality Gram matrix kernel for TRN2.

out[i, j] = 1.0 if Z[i] == Z[j] else 0.0, Z: [16384] int32 labels in [0, 64).

Row-parallel across 8 NeuronCores: core i computes rows [i*2048, (i+1)*2048).
Per core: DVE tensor_scalar(is_equal) compares a partition-broadcast copy of Z
(f32) against the per-partition row label, producing [128, CHUNK] f32 tiles
streamed to DRAM with large HWDGE DMAs. The kernel is output-write bound
(~128 MiB/core); DVE compute (~138 us) hides under the HBM writes (~370 us).
"""

import numpy as np

import concourse.tile as tile
from concourse import bacc, mybir
from concourse.bass_utils import run_bass_kernel_spmd

N = 16384          # number of labels / output dim
M = 8              # cores
RPC = N // M       # 2048 rows per core
P = 128            # SBUF partitions
T = RPC // P       # 16 row tiles per core
CHUNK = 8192       # output column chunk per DVE op / DMA store
NCH = N // CHUNK

_NC_CACHE = None


def _build_nc():
    nc = bacc.Bacc("TRN2", target_bir_lowering=False, debug=False, num_devices=M)
    # uint8 labels (values 0-63): 2 MiB broadcast read instead of 8 MiB f32
    zb = nc.dram_tensor("zb", [P, N], mybir.dt.uint8, kind="ExternalInput").ap()
    zr = nc.dram_tensor("zr", [P, T], mybir.dt.float32, kind="ExternalInput").ap()
    out = nc.dram_tensor("out", [RPC, N], mybir.dt.float32, kind="ExternalOutput").ap()

    FIRST = 2048   # small first tile so the store stream starts early
    REST = N - FIRST  # 14336

    with tile.TileContext(nc) as tc:
        with tc.tile_pool(name="zp", bufs=2) as zp, \
             tc.tile_pool(name="ep", bufs=4) as ep, \
             tc.tile_pool(name="e0p", bufs=1) as e0p, \
             tc.tile_pool(name="rp", bufs=1) as rp:
            zrt = rp.tile([P, T], mybir.dt.float32)
            # zr on the scalar HWDGE ring: overlaps with z loads on sync
            nc.scalar.dma_start(zrt[:], zr[:, :])
            za = zp.tile([P, FIRST], mybir.dt.uint8, tag="za")
            nc.sync.dma_start(za[:], zb[:, 0:FIRST])
            zb2 = zp.tile([P, REST], mybir.dt.uint8, tag="zb2")
            nc.sync.dma_start(zb2[:], zb[:, FIRST:N])

            def cmp(dst_ap, src_ap, t):
                nc.vector.tensor_scalar(
                    dst_ap, src_ap, zrt[:, t:t + 1], None,
                    mybir.AluOpType.is_equal,
                )

            # t=0: 1 + 4 + 3 MiB stores, first one tiny to prime the pipeline
            e0 = e0p.tile([P, FIRST], mybir.dt.float32)
            cmp(e0[:], za[:], 0)
            nc.sync.dma_start(out[0:P, 0:FIRST], e0[:])
            eA = ep.tile([P, CHUNK], mybir.dt.float32, tag="eq")
            cmp(eA[:], zb2[:, 0:CHUNK], 0)
            nc.sync.dma_start(out[0:P, FIRST:FIRST + CHUNK], eA[:])
            eB = ep.tile([P, CHUNK], mybir.dt.float32, tag="eq")
            cmp(eB[:, 0:REST - CHUNK], zb2[:, CHUNK:REST], 0)
            nc.sync.dma_start(out[0:P, FIRST + CHUNK:N], eB[:, 0:REST - CHUNK])

            # t>=1: two 4 MiB stores per row tile (32 KiB packets pace the
            # 16 SDMA engines evenly; 64 KiB-packet stores leave a straggler)
            for t in range(1, T):
                r0, r1 = t * P, (t + 1) * P
                eq1 = ep.tile([P, CHUNK], mybir.dt.float32, tag="eq")
                cmp(eq1[:, 0:FIRST], za[:], t)
                cmp(eq1[:, FIRST:CHUNK], zb2[:, 0:CHUNK - FIRST], t)
                nc.sync.dma_start(out[r0:r1, 0:CHUNK], eq1[:])
                eq2 = ep.tile([P, CHUNK], mybir.dt.float32, tag="eq")
                cmp(eq2[:], zb2[:, CHUNK - FIRST:REST], t)
                nc.sync.dma_start(out[r0:r1, CHUNK:N], eq2[:])
    nc.compile()
    return nc


def _get_nc():
    global _NC_CACHE
    if _NC_CACHE is None:
        _NC_CACHE = _build_nc()
    return _NC_CACHE


def _in_maps(Z: np.ndarray) -> list[dict[str, np.ndarray]]:
    zflat = Z.reshape(-1)
    zb = np.ascontiguousarray(
        np.broadcast_to(zflat.astype(np.uint8)[None, :], (P, N))
    )
    zf = zflat.astype(np.float32)
    maps = []
    for i in range(M):
        # zr[p, t] = label of row (i*RPC + t*P + p)
        zr_i = np.ascontiguousarray(zf[i * RPC:(i + 1) * RPC].reshape(T, P).T)
        maps.append({"zb": zb, "zr": zr_i})
    return maps


def kernel(Z: np.ndarray, **_ignored) -> np.ndarray:
    Z = np.asarray(Z).reshape(-1)
    assert Z.shape == (N,), Z.shape
    nc = _get_nc()
    res = run_bass_kernel_spmd(nc, _in_maps(Z), list(range(M)))
    return np.concatenate([res.results[i]["out"] for i in range(M)], axis=0)
